# revision 42
# baseline (speedup 1.0000x reference)
"""MultiHeadAttention Trainium2 kernel.

Problem: B=4, T=2048, d_model=512, H=8 heads, d_k=64 (fp32 reference).

Sharding: 8 cores = 4 batches x 2 head-groups (4 heads each).
Each core computes, for its (batch, head-group):
    q/k/v projections -> attention -> partial output projection
Host sums the two head-group partials per batch and adds the output bias.

Measured HW model (NTFF): a matmul costs ~1 cycle per FLATTENED moving
column regardless of dtype (fp8 DoubleRow included), so the kernel
minimizes total moving columns and spends its dtype budget on accuracy:
  - Projections and scores in plain bf16 (contraction via PSUM
    accumulation); scores S^T [tk=128, tq=512] f32 in PSUM.
  - attnV keeps fp8e4 DoubleRow with tk-tile-paired es [128, 2, 512]:
    same streaming cost as bf16 but half the LDWEIGHTS, and the 256-deep
    contraction halves PSUM accumulation passes.  V is augmented with a
    ones column so PSUM row 64 accumulates softmax denominators free
    (padded to 96 columns: DR stationary must be 32-aligned).
  - exp() is the co-bottleneck (16.8M f32 PSUM reads/core; GPSIMD can't
    touch PSUM): split between ACT (native Exp, fp8 out) and DVE running
    a Schraudolph bit-trick exp that emits fp8e4 BITS directly:
      fp8_bits = int8(s * (8/ln2)/8 + 56.16)
    Its constant-offset error is a uniform scale on es and cancels in
    the softmax normalization.
  - Normalize: reciprocal of PSUM row 64 (both heads of a pair share one
    [96, 2, 512] accumulator -> 1024-col vector ops), partition-broadcast
    via DRAM bounce, multiply into bf16 attnT; out-projection (bf16) of
    block qb is deferred into block qb+1 to hide the normalize latency.
"""

import functools

import numpy as np

B, T, D = 4, 2048, 512
H, DK = 8, 64
P = 128
KC = D // P            # 4 contraction chunks of x
HG = 2                 # head groups (cores per batch)
HPG = H // HG          # 4 heads per group
GC = HPG * DK          # 256 channels per group
QB = 512               # tq block
NQB = T // QB          # 4
NTK = T // P           # 16 tk tiles
NTP = NTK // 2         # 8 tk tile-pairs
NCORES = 8

MC = 2                 # q/k m-chunks (head pairs per group)
EXPSC = 1.0 / 8.0      # exp input scale: 1/sqrt(d_k)
# Schraudolph fp8e4m3 bit-trick constants (applied to raw s = q.k):
#   bits = s * (8/ln2) * EXPSC + (7*8 + 0.5 - 0.344)
SCH_A = (8.0 / float(np.log(2.0))) * EXPSC
SCH_B = 56.156

CFG = dict(
    exp_pat="AADADADAD",   # A = ACT native Exp, D = DVE schraudolph
)

# kept for test.py compat (unused)
DTCFG = dict(A="fp8", B="fp8", C="fp8", D="bf16")


@functools.cache
def _build(exp_pat):
    import concourse.bass as bass
    import concourse.mybir as mybir
    import concourse.tile as tile
    from concourse import bacc
    from concourse.bass import ds, ts

    f32 = mybir.dt.float32
    bf16 = mybir.dt.bfloat16
    fp8 = mybir.dt.float8e4
    i8 = mybir.dt.int8
    Exp = mybir.ActivationFunctionType.Exp
    Identity = mybir.ActivationFunctionType.Identity
    DR = mybir.MatmulPerfMode.DoubleRow
    mult = mybir.AluOpType.mult
    add = mybir.AluOpType.add


    nc = bacc.Bacc()
    xT_d = nc.dram_tensor("xT", [D, T], bf16, kind="ExternalInput")
    wqT_d = nc.dram_tensor("wqT", [D, GC], bf16, kind="ExternalInput")
    wkT_d = nc.dram_tensor("wkT", [D, GC], bf16, kind="ExternalInput")
    wvT_d = nc.dram_tensor("wvT", [D, GC], bf16, kind="ExternalInput")
    bqk_d = nc.dram_tensor("bqk", [P, 2, MC], f32, kind="ExternalInput")
    bv_d = nc.dram_tensor("bv", [GC], f32, kind="ExternalInput")
    woT_d = nc.dram_tensor("woT", [HPG, DK, D], bf16, kind="ExternalInput")
    part_d = nc.dram_tensor("part", [T, D], f32, kind="ExternalOutput")
    # DRAM bounce buffers for the softmax denominators: raw rowsums out,
    # reciprocals back (transposed through [128, 8] so one DVE reciprocal
    # covers 1024 values across all partitions)
    nsc_d = nc.dram_tensor("nscratch", [8, 2, 512], f32)
    nscr_d = nc.dram_tensor("nscratchr", [8, 2, 512], f32)

    part_t = part_d.rearrange("(n p) o -> n p o", p=P)

    with tile.TileContext(nc) as tc:
        with (
            tc.tile_pool(name="const", bufs=1) as const,
            tc.tile_pool(name="big", bufs=1) as big,
        ):
            # ---- constant / persistent SBUF tensors ----
            # weights first, then x in tq-quarters: the first projection
            # chunk only needs wq + the first quarter of x
            # DMA issue order matches first-use order: the first projection
            # chunk needs only wq + the first half of x
            wqT = const.tile([P, KC, GC], bf16, tag="wqT")
            nc.sync.dma_start(out=wqT, in_=wqT_d.rearrange("(a p) m -> p a m", p=P))
            xT = const.tile([P, KC, T], bf16)
            xT_r = xT_d.rearrange("(a p) t -> p a t", p=P)
            for tq in range(2):
                nc.sync.dma_start(
                    out=xT[:, :, ts(tq, 512)], in_=xT_r[:, :, ts(tq, 512)]
                )
            bqk = const.tile([P, 2, MC], f32, tag="bqk")
            nc.sync.dma_start(out=bqk, in_=bqk_d[:])
            wkT = const.tile([P, KC, GC], bf16, tag="wkT")
            nc.sync.dma_start(out=wkT, in_=wkT_d.rearrange("(a p) m -> p a m", p=P))
            for tq in range(2, 4):
                nc.sync.dma_start(
                    out=xT[:, :, ts(tq, 512)], in_=xT_r[:, :, ts(tq, 512)]
                )
            wvT = const.tile([P, KC, GC], bf16, tag="wvT")
            nc.sync.dma_start(out=wvT, in_=wvT_d.rearrange("(a p) m -> p a m", p=P))
            woTs = const.tile([DK, HPG, D], bf16, tag="woT")
            nc.sync.dma_start(out=woTs, in_=woT_d.rearrange("h p o -> p h o"))
            # v-bias broadcast across all 128 partitions
            bvb = const.tile([P, GC], f32, tag="bvb")
            bv_bcast = bass.AP(
                tensor=bv_d[:].tensor,
                offset=bv_d[:].offset,
                ap=[[0, P]] + list(bv_d[:].ap),
            )
            nc.sync.dma_start(out=bvb, in_=bv_bcast)

            # q/k bf16, head h: partitions 64*(h%2)..+64, m-chunk h//2
            qT = big.tile([P, MC, T], bf16, tag="qT")
            kT = big.tile([P, MC, T], bf16, tag="kT")
            # v augmented with ones column; [tk%128, tk-tile-pair, pair-elt,
            # head, dv(64)+one]
            v_aug = big.tile([P, NTP, 2, HPG, 96], fp8, tag="v_aug")
            # attn (normalized, bf16) per (head-pair, qb): [dv, head, tq]
            attnT = [
                [
                    big.tile(
                        [DK, 2, QB], bf16, tag=f"attnT{hp}_{q}",
                        name=f"attnT{hp}_{q}"
                    )
                    for q in range(NQB)
                ]
                for hp in range(2)
            ]

            # col 64 of v_aug = 1 -> PSUM row 64 = softmax denominators;
            # cols 65:96 = 0 (DoubleRow stationary width must be 32-aligned)
            nc.vector.memset(v_aug[:, :, :, :, 64:65], 1.0)
            nc.vector.memset(v_aug[:, :, :, :, 65:96], 0.0)

            # ---- projections ----
            with (
                tc.tile_pool(name="psproj", bufs=3, space="PSUM") as psp,
            ):
                bctr = 0
                for pi, (wt, dst) in enumerate(((wqT, qT), (wkT, kT))):
                    for m in range(MC):
                        for ntp in range(2):
                            ps = psp.tile([P, 2, 512], f32, tag="pq")
                            for n2 in range(2):
                                nt = 2 * ntp + n2
                                for kc in range(KC):
                                    nc.tensor.matmul(
                                        ps[:, n2, :],
                                        wt[:, kc, ts(m, P)],
                                        xT[:, kc, ts(nt, 512)],
                                        start=(kc == 0),
                                        stop=(kc == KC - 1),
                                    )
                            dst_ap = dst[:, m, ts(ntp, 1024)].rearrange(
                                "p (n q) -> p n q", n=2
                            )
                            if bctr % 2 == 0:
                                nc.vector.tensor_scalar_add(
                                    dst_ap, ps, bqk[:, pi, m : m + 1]
                                )
                            else:
                                nc.scalar.activation(
                                    dst_ap,
                                    ps,
                                    Identity,
                                    bias=bqk[:, pi, m : m + 1],
                                )
                            bctr += 1

            # ---- attention + output projection ----
            with (
                tc.tile_pool(name="pssc", bufs=4, space="PSUM") as pssc,
                tc.tile_pool(name="psoa", bufs=2, space="PSUM") as psoa,
                tc.tile_pool(name="esp", bufs=3) as esp,
                tc.tile_pool(name="normp", bufs=2) as normp,
                tc.tile_pool(name="outp", bufs=3) as outp,
            ):
                expctr = [0]

                def emit_vproj(t):
                    # v projection interleaved into qb0's score stream; the
                    # PSUM tile borrows half an "sc" slot
                    ps = pssc.tile([P, 512], f32, tag="sc", name="pv")
                    for kc in range(KC):
                        nc.tensor.matmul(
                            ps[:, 0:GC],
                            xT[:, kc, ts(t, P)],
                            wvT[:, kc, :],
                            start=(kc == 0),
                            stop=(kc == KC - 1),
                        )
                    tp_, j_ = divmod(t, 2)
                    nc.vector.tensor_add(
                        v_aug[:, tp_, j_, :, 0:64],
                        ps[:, 0:GC].rearrange("p (h c) -> p h c", h=HPG),
                        bvb.rearrange("p (h c) -> p h c", h=HPG),
                    )

                def emit_exp(es_slice, sc):
                    eng = exp_pat[expctr[0] % len(exp_pat)]
                    expctr[0] += 1
                    if eng == "A":
                        nc.scalar.activation(es_slice, sc, Exp, scale=EXPSC)
                    else:
                        nc.vector.tensor_scalar(
                            es_slice.bitcast(i8),
                            sc,
                            SCH_A,
                            SCH_B,
                            op0=mult,
                            op1=add,
                        )

                def emit_outproj_tt(qb, tt):
                    po = pssc.tile([P, 512], f32, tag="sc", name="po")
                    for hp in range(2):
                        for hh in range(2):
                            nc.tensor.matmul(
                                po,
                                attnT[hp][qb][:, hh, ts(tt, P)],
                                woTs[:, 2 * hp + hh, :],
                                start=(hp == 0 and hh == 0),
                                stop=(hp == 1 and hh == 1),
                            )
                    ostage = outp.tile([P, 512], f32, tag="ostage")
                    if (qb + tt) % 2 == 0:
                        nc.scalar.copy(ostage, po)
                    else:
                        nc.vector.tensor_copy(ostage, po)
                    nc.sync.dma_start(out=part_t[qb * 4 + tt], in_=ostage)

                def attn_v(oacc, es, hp, tp):
                    for hh in range(2):
                        nc.tensor.matmul(
                            oacc[:, hh, :],
                            v_aug[:, tp, :, 2 * hp + hh, 0:96],
                            es[hh],
                            start=(tp == 0),
                            stop=(tp == NTP - 1),
                            perf_mode=DR,
                        )

                for qb in range(NQB):
                    for hp in range(2):
                        if qb == 0 and hp == 0:
                            for t in range(4):
                                emit_vproj(t)
                        oacc = psoa.tile([96, 2, 512], f32, tag="oacc")
                        prev_es = prev2_es = None
                        for tp in range(NTP):
                            if qb == 0 and hp == 0 and tp < 6:
                                emit_vproj(2 * tp + 4)
                                emit_vproj(2 * tp + 5)
                            es = [
                                esp.tile(
                                    [P, 2, 512], fp8, tag=f"es{hh}", name=f"es{hh}"
                                )
                                for hh in range(2)
                            ]
                            for j in range(2):
                                t = 2 * tp + j
                                for hh in range(2):
                                    h = 2 * hp + hh
                                    hb, m = 64 * (h % 2), h // 2
                                    sc = pssc.tile([P, 512], f32, tag="sc")
                                    nc.tensor.matmul(
                                        sc,
                                        kT[ds(hb, 64), m, ts(t, P)],
                                        qT[ds(hb, 64), m, ts(qb, 512)],
                                        start=True,
                                        stop=True,
                                    )
                                    emit_exp(es[hh][:, j, :], sc)
                            # attnV deferred two tk-pairs so the in-order PE
                            # queue never waits on the exp engines
                            if tp > 1:
                                attn_v(oacc, prev2_es, hp, tp - 2)
                            prev2_es = prev_es
                            prev_es = es
                            if qb > 0 and hp == 1 and tp in (0, 1, 2, 3):
                                emit_outproj_tt(qb - 1, tp)
                        attn_v(oacc, prev2_es, hp, NTP - 2)
                        attn_v(oacc, prev_es, hp, NTP - 1)
                        # normalize both heads of the pair: copy rowsums out,
                        # reciprocal in a [128, 8] transposed layout (one
                        # cheap full-width DVE op), broadcast back
                        slot = qb * 2 + hp
                        rst = normp.tile([P, 2, 512], f32, tag="rst")
                        nc.vector.tensor_copy(
                            rst[64:65, :, :], oacc[64:65, :, :]
                        )
                        nc.sync.dma_start(
                            out=nsc_d[slot : slot + 1, :, :],
                            in_=rst[64:65, :, :],
                        )
                        rs_t = normp.tile([P, 8], f32, tag="rs_t")
                        nc.sync.dma_start(
                            out=rs_t,
                            in_=nsc_d[slot].rearrange("a (p c) -> (a p) c", p=P),
                        )
                        rsr = normp.tile([P, 8], f32, tag="rsr")
                        nc.vector.reciprocal(rsr, rs_t)
                        nc.sync.dma_start(
                            out=nscr_d[slot].rearrange("a (p c) -> (a p) c", p=P),
                            in_=rsr,
                        )
                        bc = normp.tile([64, 2, 512], f32, tag="bc")
                        for hh in range(2):
                            nsrow = nscr_d[slot, hh, :]
                            bc_src = bass.AP(
                                tensor=nsrow.tensor,
                                offset=nsrow.offset,
                                ap=[[0, 64]] + list(nsrow.ap),
                            )
                            nc.sync.dma_start(out=bc[:, hh, :], in_=bc_src)
                        nc.vector.tensor_mul(
                            attnT[hp][qb], oacc[0:64, :, :], bc
                        )
                for tt in range(4):
                    emit_outproj_tt(NQB - 1, tt)

    nc.compile()
    return nc


def _host_prep(x, wq, bq, wk, bk, wv, bv, wo):
    """Build the 8 per-core input maps."""
    import ml_dtypes

    b16 = ml_dtypes.bfloat16
    x = np.asarray(x, dtype=np.float32)
    maps = []
    per_hg = {}
    for hg in range(HG):
        rows = slice(hg * GC, (hg + 1) * GC)
        wqT = np.ascontiguousarray(np.asarray(wq)[rows, :].T.astype(b16))
        wkT = np.ascontiguousarray(np.asarray(wk)[rows, :].T.astype(b16))
        wvT = np.ascontiguousarray(np.asarray(wv)[rows, :].T.astype(b16))
        bq_r = np.asarray(bq, dtype=np.float32)[rows].reshape(MC, P).T
        bk_r = np.asarray(bk, dtype=np.float32)[rows].reshape(MC, P).T
        bqk = np.ascontiguousarray(
            np.stack([bq_r, bk_r], axis=1), dtype=np.float32
        )  # [128, 2, MC]
        bv_s = np.ascontiguousarray(np.asarray(bv)[rows], dtype=np.float32)
        woT = np.ascontiguousarray(
            np.asarray(wo, dtype=np.float32).T[rows, :]
            .reshape(HPG, DK, D)
            .astype(b16)
        )
        per_hg[hg] = dict(wqT=wqT, wkT=wkT, wvT=wvT, bqk=bqk, bv=bv_s, woT=woT)
    for b in range(B):
        xT = np.ascontiguousarray(x[b].T.astype(b16))
        for hg in range(HG):
            maps.append(dict(xT=xT, **per_hg[hg]))
    return maps


def kernel(x, wq, bq, wk, bk, wv, bv, wo, bo, _run_opts=None):
    from concourse.bass_utils import run_bass_kernel_spmd

    nc = _build(CFG["exp_pat"])
    in_maps = _host_prep(x, wq, bq, wk, bk, wv, bv, wo)
    opts = _run_opts or {}
    res = run_bass_kernel_spmd(nc, in_maps, core_ids=list(range(NCORES)), **opts)
    bo = np.asarray(bo, dtype=np.float32)
    out = np.empty((B, T, D), dtype=np.float32)
    for b in range(B):
        out[b] = res.results[2 * b]["part"] + res.results[2 * b + 1]["part"] + bo
    if opts:
        kernel.last_results = res
    return out


# revision 43
# speedup vs baseline: 1.0944x; 1.0944x over previous
"""MultiHeadAttention Trainium2 kernel.

Problem: B=4, T=2048, d_model=512, H=8 heads, d_k=64 (fp32 reference).

Sharding: 8 cores = 4 batches x 2 head-groups (4 heads each).
Each core computes, for its (batch, head-group):
    q/k/v projections -> attention -> partial output projection
Host sums the two head-group partials per batch and adds the output bias.

Measured HW model (NTFF): a matmul costs ~1 cycle per FLATTENED moving
column regardless of dtype (fp8 DoubleRow included), so the kernel
minimizes total moving columns and spends its dtype budget on accuracy:
  - Projections and scores in plain bf16 (contraction via PSUM
    accumulation); scores S^T [tk=128, tq=512] f32 in PSUM.
  - attnV keeps fp8e4 DoubleRow with tk-tile-paired es [128, 2, 512]:
    same streaming cost as bf16 but half the LDWEIGHTS, and the 256-deep
    contraction halves PSUM accumulation passes.  V is augmented with a
    ones column so PSUM row 64 accumulates softmax denominators free
    (padded to 96 columns: DR stationary must be 32-aligned).
  - exp() is the co-bottleneck (16.8M f32 PSUM reads/core; GPSIMD can't
    touch PSUM): split between ACT (native Exp, fp8 out) and DVE running
    a Schraudolph bit-trick exp that emits fp8e4 BITS directly:
      fp8_bits = int8(s * (8/ln2)/8 + 56.16)
    Its constant-offset error is a uniform scale on es and cancels in
    the softmax normalization.
  - Normalize: reciprocal of PSUM row 64 (both heads of a pair share one
    [96, 2, 512] accumulator -> 1024-col vector ops), partition-broadcast
    via DRAM bounce, multiply into bf16 attnT; out-projection (bf16) of
    block qb is deferred into block qb+1 to hide the normalize latency.
"""

import functools

import numpy as np

B, T, D = 4, 2048, 512
H, DK = 8, 64
P = 128
KC = D // P            # 4 contraction chunks of x
HG = 2                 # head groups (cores per batch)
HPG = H // HG          # 4 heads per group
GC = HPG * DK          # 256 channels per group
QB = 512               # tq block
NQB = T // QB          # 4
NTK = T // P           # 16 tk tiles
NTP = NTK // 2         # 8 tk tile-pairs
NCORES = 8

MC = 2                 # q/k m-chunks (head pairs per group)
EXPSC = 1.0 / 8.0      # exp input scale: 1/sqrt(d_k)
# Schraudolph fp8e4m3 bit-trick constants (applied to raw s = q.k):
#   bits = s * (8/ln2) * EXPSC + (7*8 + 0.5 - 0.344)
SCH_A = (8.0 / float(np.log(2.0))) * EXPSC
SCH_B = 56.156

CFG = dict(
    exp_pat="AADADADAD",   # A = ACT native Exp, D = DVE schraudolph
)

# kept for test.py compat (unused)
DTCFG = dict(A="fp8", B="fp8", C="fp8", D="bf16")


@functools.cache
def _build(exp_pat):
    import concourse.bass as bass
    import concourse.mybir as mybir
    import concourse.tile as tile
    from concourse import bacc
    from concourse.bass import ds, ts

    f32 = mybir.dt.float32
    bf16 = mybir.dt.bfloat16
    fp8 = mybir.dt.float8e4
    i8 = mybir.dt.int8
    Exp = mybir.ActivationFunctionType.Exp
    Identity = mybir.ActivationFunctionType.Identity
    DR = mybir.MatmulPerfMode.DoubleRow
    mult = mybir.AluOpType.mult
    add = mybir.AluOpType.add


    nc = bacc.Bacc()
    xT_d = nc.dram_tensor("xT", [D, T], bf16, kind="ExternalInput")
    wqT_d = nc.dram_tensor("wqT", [D, GC], bf16, kind="ExternalInput")
    wkT_d = nc.dram_tensor("wkT", [D, GC], bf16, kind="ExternalInput")
    wvT_d = nc.dram_tensor("wvT", [D, GC], bf16, kind="ExternalInput")
    bqk_d = nc.dram_tensor("bqk", [P, 2, MC], f32, kind="ExternalInput")
    bv_d = nc.dram_tensor("bv", [GC], f32, kind="ExternalInput")
    woT_d = nc.dram_tensor("woT", [HPG, DK, D], bf16, kind="ExternalInput")
    part_d = nc.dram_tensor("part", [T, D], f32, kind="ExternalOutput")
    # DRAM bounce buffers for the softmax denominators: raw rowsums out,
    # reciprocals back (transposed through [128, 8] so one DVE reciprocal
    # covers 1024 values across all partitions)
    nsc_d = nc.dram_tensor("nscratch", [8, 2, 512], f32)
    nscr_d = nc.dram_tensor("nscratchr", [8, 2, 512], f32)

    part_t = part_d.rearrange("(n p) o -> n p o", p=P)

    with tile.TileContext(nc) as tc:
        with (
            tc.tile_pool(name="const", bufs=1) as const,
            tc.tile_pool(name="big", bufs=1) as big,
        ):
            # ---- constant / persistent SBUF tensors ----
            # weights first, then x in tq-quarters: the first projection
            # chunk only needs wq + the first quarter of x
            # DMA issue order matches first-use order: the first projection
            # chunk needs only wq + the first half of x
            wqT = const.tile([P, KC, GC], bf16, tag="wqT")
            nc.sync.dma_start(out=wqT, in_=wqT_d.rearrange("(a p) m -> p a m", p=P))
            xT = const.tile([P, KC, T], bf16)
            xT_r = xT_d.rearrange("(a p) t -> p a t", p=P)
            for tq in range(2):
                nc.sync.dma_start(
                    out=xT[:, :, ts(tq, 512)], in_=xT_r[:, :, ts(tq, 512)]
                )
            bqk = const.tile([P, 2, MC], f32, tag="bqk")
            nc.sync.dma_start(out=bqk, in_=bqk_d[:])
            wkT = const.tile([P, KC, GC], bf16, tag="wkT")
            nc.sync.dma_start(out=wkT, in_=wkT_d.rearrange("(a p) m -> p a m", p=P))
            for tq in range(2, 4):
                nc.sync.dma_start(
                    out=xT[:, :, ts(tq, 512)], in_=xT_r[:, :, ts(tq, 512)]
                )
            wvT = const.tile([P, KC, GC], bf16, tag="wvT")
            nc.sync.dma_start(out=wvT, in_=wvT_d.rearrange("(a p) m -> p a m", p=P))
            woTs = const.tile([DK, HPG, D], bf16, tag="woT")
            nc.sync.dma_start(out=woTs, in_=woT_d.rearrange("h p o -> p h o"))
            # v-bias broadcast across all 128 partitions
            bvb = const.tile([P, GC], f32, tag="bvb")
            bv_bcast = bass.AP(
                tensor=bv_d[:].tensor,
                offset=bv_d[:].offset,
                ap=[[0, P]] + list(bv_d[:].ap),
            )
            nc.sync.dma_start(out=bvb, in_=bv_bcast)

            # q/k bf16, head h: partitions 64*(h%2)..+64, m-chunk h//2
            qT = big.tile([P, MC, T], bf16, tag="qT")
            kT = big.tile([P, MC, T], bf16, tag="kT")
            # v augmented with ones column; [tk%128, tk-tile-pair, pair-elt,
            # head, dv(64)+one]
            v_aug = big.tile([P, NTP, 2, HPG, 96], fp8, tag="v_aug")
            # attn (normalized, bf16) per (head-pair, qb): [dv, head, tq]
            attnT = [
                [
                    big.tile(
                        [DK, 2, QB], bf16, tag=f"attnT{hp}_{q}",
                        name=f"attnT{hp}_{q}"
                    )
                    for q in range(NQB)
                ]
                for hp in range(2)
            ]

            # col 64 of v_aug = 1 -> PSUM row 64 = softmax denominators;
            # cols 65:96 = 0 (DoubleRow stationary width must be 32-aligned)
            nc.vector.memset(v_aug[:, :, :, :, 64:65], 1.0)
            nc.vector.memset(v_aug[:, :, :, :, 65:96], 0.0)

            # ---- projections ----
            with (
                tc.tile_pool(name="psproj", bufs=3, space="PSUM") as psp,
                tc.tile_pool(name="psv", bufs=2, space="PSUM") as psv,
            ):
                bctr = 0
                for pi, (wt, dst) in enumerate(((wqT, qT), (wkT, kT))):
                    for m in range(MC):
                        for ntp in range(2):
                            ps = psp.tile([P, 2, 512], f32, tag="pq")
                            for n2 in range(2):
                                nt = 2 * ntp + n2
                                for kc in range(KC):
                                    nc.tensor.matmul(
                                        ps[:, n2, :],
                                        wt[:, kc, ts(m, P)],
                                        xT[:, kc, ts(nt, 512)],
                                        start=(kc == 0),
                                        stop=(kc == KC - 1),
                                    )
                            dst_ap = dst[:, m, ts(ntp, 1024)].rearrange(
                                "p (n q) -> p n q", n=2
                            )
                            if bctr % 2 == 0:
                                nc.vector.tensor_scalar_add(
                                    dst_ap, ps, bqk[:, pi, m : m + 1]
                                )
                            else:
                                nc.scalar.activation(
                                    dst_ap,
                                    ps,
                                    Identity,
                                    bias=bqk[:, pi, m : m + 1],
                                )
                            bctr += 1
                for t in range(NTK):
                    ps = psv.tile([P, GC], f32, tag="pv")
                    for kc in range(KC):
                        nc.tensor.matmul(
                            ps,
                            xT[:, kc, ts(t, P)],
                            wvT[:, kc, :],
                            start=(kc == 0),
                            stop=(kc == KC - 1),
                        )
                    tp, j = divmod(t, 2)
                    nc.vector.tensor_add(
                        v_aug[:, tp, j, :, 0:64],
                        ps.rearrange("p (h c) -> p h c", h=HPG),
                        bvb.rearrange("p (h c) -> p h c", h=HPG),
                    )

            # ---- attention + output projection ----
            with (
                tc.tile_pool(name="pssc", bufs=4, space="PSUM") as pssc,
                tc.tile_pool(name="psoa", bufs=2, space="PSUM") as psoa,
                tc.tile_pool(name="esp", bufs=3) as esp,
                tc.tile_pool(name="normp", bufs=2) as normp,
                tc.tile_pool(name="outp", bufs=3) as outp,
            ):
                expctr = [0]

                def emit_exp(es_slice, sc):
                    eng = exp_pat[expctr[0] % len(exp_pat)]
                    expctr[0] += 1
                    if eng == "A":
                        nc.scalar.activation(es_slice, sc, Exp, scale=EXPSC)
                    else:
                        nc.vector.tensor_scalar(
                            es_slice.bitcast(i8),
                            sc,
                            SCH_A,
                            SCH_B,
                            op0=mult,
                            op1=add,
                        )

                def emit_outproj_tt(qb, tt):
                    po = pssc.tile([P, 512], f32, tag="sc", name="po")
                    for hp in range(2):
                        for hh in range(2):
                            nc.tensor.matmul(
                                po,
                                attnT[hp][qb][:, hh, ts(tt, P)],
                                woTs[:, 2 * hp + hh, :],
                                start=(hp == 0 and hh == 0),
                                stop=(hp == 1 and hh == 1),
                            )
                    ostage = outp.tile([P, 512], f32, tag="ostage")
                    if (qb + tt) % 2 == 0:
                        nc.scalar.copy(ostage, po)
                    else:
                        nc.vector.tensor_copy(ostage, po)
                    nc.sync.dma_start(out=part_t[qb * 4 + tt], in_=ostage)

                def attn_v(oacc, es, hp, tp):
                    for hh in range(2):
                        nc.tensor.matmul(
                            oacc[:, hh, :],
                            v_aug[:, tp, :, 2 * hp + hh, 0:96],
                            es[hh],
                            start=(tp == 0),
                            stop=(tp == NTP - 1),
                            perf_mode=DR,
                        )

                for qb in range(NQB):
                    for hp in range(2):
                        oacc = psoa.tile([96, 2, 512], f32, tag="oacc")
                        prev_es = prev2_es = None
                        for tp in range(NTP):
                            es = [
                                esp.tile(
                                    [P, 2, 512], fp8, tag=f"es{hh}", name=f"es{hh}"
                                )
                                for hh in range(2)
                            ]
                            for j in range(2):
                                t = 2 * tp + j
                                for hh in range(2):
                                    h = 2 * hp + hh
                                    hb, m = 64 * (h % 2), h // 2
                                    sc = pssc.tile([P, 512], f32, tag="sc")
                                    nc.tensor.matmul(
                                        sc,
                                        kT[ds(hb, 64), m, ts(t, P)],
                                        qT[ds(hb, 64), m, ts(qb, 512)],
                                        start=True,
                                        stop=True,
                                    )
                                    emit_exp(es[hh][:, j, :], sc)
                            # attnV deferred two tk-pairs so the in-order PE
                            # queue never waits on the exp engines
                            if tp > 1:
                                attn_v(oacc, prev2_es, hp, tp - 2)
                            prev2_es = prev_es
                            prev_es = es
                            if qb > 0 and hp == 1 and tp in (0, 1, 2, 3):
                                emit_outproj_tt(qb - 1, tp)
                        attn_v(oacc, prev2_es, hp, NTP - 2)
                        attn_v(oacc, prev_es, hp, NTP - 1)
                        # normalize both heads of the pair: copy rowsums out,
                        # reciprocal in a [128, 8] transposed layout (one
                        # cheap full-width DVE op), broadcast back
                        slot = qb * 2 + hp
                        rst = normp.tile([P, 2, 512], f32, tag="rst")
                        nc.vector.tensor_copy(
                            rst[64:65, :, :], oacc[64:65, :, :]
                        )
                        nc.sync.dma_start(
                            out=nsc_d[slot : slot + 1, :, :],
                            in_=rst[64:65, :, :],
                        )
                        rs_t = normp.tile([P, 8], f32, tag="rs_t")
                        nc.sync.dma_start(
                            out=rs_t,
                            in_=nsc_d[slot].rearrange("a (p c) -> (a p) c", p=P),
                        )
                        rsr = normp.tile([P, 8], f32, tag="rsr")
                        nc.vector.reciprocal(rsr, rs_t)
                        nc.sync.dma_start(
                            out=nscr_d[slot].rearrange("a (p c) -> (a p) c", p=P),
                            in_=rsr,
                        )
                        bc = normp.tile([64, 2, 512], f32, tag="bc")
                        for hh in range(2):
                            nsrow = nscr_d[slot, hh, :]
                            bc_src = bass.AP(
                                tensor=nsrow.tensor,
                                offset=nsrow.offset,
                                ap=[[0, 64]] + list(nsrow.ap),
                            )
                            nc.sync.dma_start(out=bc[:, hh, :], in_=bc_src)
                        nc.vector.tensor_mul(
                            attnT[hp][qb], oacc[0:64, :, :], bc
                        )
                for tt in range(4):
                    emit_outproj_tt(NQB - 1, tt)

    nc.compile()
    return nc


def _host_prep(x, wq, bq, wk, bk, wv, bv, wo):
    """Build the 8 per-core input maps."""
    import ml_dtypes

    b16 = ml_dtypes.bfloat16
    x = np.asarray(x, dtype=np.float32)
    maps = []
    per_hg = {}
    for hg in range(HG):
        rows = slice(hg * GC, (hg + 1) * GC)
        wqT = np.ascontiguousarray(np.asarray(wq)[rows, :].T.astype(b16))
        wkT = np.ascontiguousarray(np.asarray(wk)[rows, :].T.astype(b16))
        wvT = np.ascontiguousarray(np.asarray(wv)[rows, :].T.astype(b16))
        bq_r = np.asarray(bq, dtype=np.float32)[rows].reshape(MC, P).T
        bk_r = np.asarray(bk, dtype=np.float32)[rows].reshape(MC, P).T
        bqk = np.ascontiguousarray(
            np.stack([bq_r, bk_r], axis=1), dtype=np.float32
        )  # [128, 2, MC]
        bv_s = np.ascontiguousarray(np.asarray(bv)[rows], dtype=np.float32)
        woT = np.ascontiguousarray(
            np.asarray(wo, dtype=np.float32).T[rows, :]
            .reshape(HPG, DK, D)
            .astype(b16)
        )
        per_hg[hg] = dict(wqT=wqT, wkT=wkT, wvT=wvT, bqk=bqk, bv=bv_s, woT=woT)
    for b in range(B):
        xT = np.ascontiguousarray(x[b].T.astype(b16))
        for hg in range(HG):
            maps.append(dict(xT=xT, **per_hg[hg]))
    return maps


def kernel(x, wq, bq, wk, bk, wv, bv, wo, bo, _run_opts=None):
    from concourse.bass_utils import run_bass_kernel_spmd

    nc = _build(CFG["exp_pat"])
    in_maps = _host_prep(x, wq, bq, wk, bk, wv, bv, wo)
    opts = _run_opts or {}
    res = run_bass_kernel_spmd(nc, in_maps, core_ids=list(range(NCORES)), **opts)
    bo = np.asarray(bo, dtype=np.float32)
    out = np.empty((B, T, D), dtype=np.float32)
    for b in range(B):
        out[b] = res.results[2 * b]["part"] + res.results[2 * b + 1]["part"] + bo
    if opts:
        kernel.last_results = res
    return out
